# revision 4
# baseline (speedup 1.0000x reference)
"""Causal self-attention Trainium2 kernel.

Reference computation (B=2, T=2048, EMB=1024, H=16 heads, D=64):
    qkv = x @ Wqkv + bqkv ; split q,k,v ; per-head causal softmax attention ;
    out = concat_heads @ Wout + bout

Sharding: 8 cores = data-parallel over batch (2) x tensor-parallel over
heads (4 heads/core).  Each core computes, for its (batch b, head shard m):
  - qkT = (x_b @ Wqk_m)^T  in d-major layout [512, 2048]  (+ bias)
  - v   = x_b @ Wv_m       in t-major layout (v bias folded into bout on host)
  - per head: S^T = k q^T (scale folded into exp), E = exp(S^T) restricted to
    the causal region, O'^T = [v | 1]^T E^T  (ones column yields softmax sums),
    normalize via reciprocal + K=1 matmul partition-broadcast
  - partial out = O_norm^T^T @ Wout_m  -> host sums the 4 TP partials per batch
All matmuls run in float32r (full-speed PE path, ~1e-4 scale-relative error).
"""

import sys

sys.path.insert(0, "/opt/trn_rl_repo")

import numpy as np

B, T, EMB = 2, 2048, 1024
H, D = 16, 64
N_CORES = 8
TP = 4  # head shards
HEADS_PER_CORE = H // TP  # 4
FSH = HEADS_PER_CORE * D  # 256 features per shard for each of q,k,v
P = 128
NG = T // 512  # 4 query groups of 512
NT = T // P  # 16 tiles of 128

_prog_cache = {}


def _build_program():
    import concourse.mybir as mybir
    import concourse.tile as tile
    from concourse import bacc

    f32 = mybir.dt.float32
    f32r = mybir.dt.float32r
    AF = mybir.ActivationFunctionType
    OP = mybir.AluOpType

    nc = bacc.Bacc("TRN2", target_bir_lowering=False)

    xT_d = nc.dram_tensor("xT", (EMB, T), f32r, kind="ExternalInput")
    wqk_d = nc.dram_tensor("wqk", (EMB, 2 * FSH), f32r, kind="ExternalInput")
    wv_d = nc.dram_tensor("wv", (EMB, FSH), f32r, kind="ExternalInput")
    wout_d = nc.dram_tensor("wout", (FSH, EMB), f32r, kind="ExternalInput")
    bqk_d = nc.dram_tensor("bqk", (P, 4), f32, kind="ExternalInput")
    tri_d = nc.dram_tensor("trimask", (P, P), f32r, kind="ExternalInput")
    out_d = nc.dram_tensor("out", (T, EMB), f32, kind="ExternalOutput")

    EK = EMB // P  # 8 contraction chunks

    with nc.allow_low_precision(
        reason="float32r tiles feed the PE fast path; fp32 accumulation in PSUM"
    ), tile.TileContext(nc) as tc:
        with (
            tc.tile_pool(name="consts", bufs=1) as consts,
            tc.tile_pool(name="qkt", bufs=1) as qkt_p,
            tc.tile_pool(name="vp", bufs=1) as v_p,
            tc.tile_pool(name="psP", bufs=2, space="PSUM") as psP,
        ):
            wqk_sb = consts.tile([P, EK, 2 * FSH], f32r)
            nc.sync.dma_start(wqk_sb[:], wqk_d.rearrange("(o p) f -> p o f", p=P))
            wv_sb = consts.tile([P, EK, FSH], f32r)
            nc.sync.dma_start(wv_sb[:], wv_d.rearrange("(o p) f -> p o f", p=P))
            wout_sb = consts.tile([P, 2, EMB], f32r)
            nc.sync.dma_start(wout_sb[:], wout_d.rearrange("(o p) f -> p o f", p=P))
            bqk_sb = consts.tile([P, 4], f32)
            nc.sync.dma_start(bqk_sb[:], bqk_d[:])
            tri_sb = consts.tile([P, P], f32r)
            nc.sync.dma_start(tri_sb[:], tri_d[:])
            ones_f32 = consts.tile([P, 1], f32)
            nc.vector.memset(ones_f32[:], 1.0)
            ones_sb = consts.tile([1, D], f32r)
            nc.vector.tensor_copy(ones_sb[:], ones_f32[0:1, 0:1].to_broadcast((1, D)))

            # qkT: d-major q|k, 4 subtiles of [128, 2048]; subtiles 0,1 = q
            # (heads 0-3 stacked 64 rows each), subtiles 2,3 = k.
            qkT = qkt_p.tile([P, 4, T], f32r)
            # v in t-major: per t-tile, 4 heads x (64 v columns + ones column)
            v_sb = v_p.tile([P, NT, HEADS_PER_CORE * (D + 1)], f32r)
            v_ones_view = v_sb.rearrange("p t (h c) -> p t h c", c=D + 1)[:, :, :, D]
            nc.vector.tensor_copy(
                v_ones_view, ones_f32[:, 0:1].to_broadcast((P, NT, HEADS_PER_CORE))
            )

            with tc.tile_pool(name="xp", bufs=1) as xp:
                xT_sb = xp.tile([P, EK, T], f32r)
                nc.sync.dma_start(xT_sb[:], xT_d.rearrange("(o p) t -> p o t", p=P))

                # ---- phase 1: qkT = Wqk^T @ xT (+bias) ----
                for f in range(4):
                    for g in range(NG):
                        ps = psP.tile([P, 512], f32, tag="pp")
                        for e in range(EK):
                            nc.tensor.matmul(
                                ps[:],
                                wqk_sb[:, e, P * f : P * (f + 1)],
                                xT_sb[:, e, 512 * g : 512 * (g + 1)],
                                start=(e == 0),
                                stop=(e == EK - 1),
                            )
                        nc.vector.tensor_scalar_add(
                            qkT[:, f, 512 * g : 512 * (g + 1)], ps[:], bqk_sb[:, f : f + 1]
                        )

                # ---- phase 2: v = x @ Wv, t-major with ones columns ----
                for t in range(NT):
                    ps = psP.tile([P, FSH], f32, tag="pp")
                    for e in range(EK):
                        nc.tensor.matmul(
                            ps[:],
                            xT_sb[:, e, P * t : P * (t + 1)],
                            wv_sb[:, e, :],
                            start=(e == 0),
                            stop=(e == EK - 1),
                        )
                    nc.vector.tensor_copy(
                        v_sb[:, t].rearrange("p (h c) -> p h c", h=HEADS_PER_CORE)[
                            :, :, :D
                        ],
                        ps[:].rearrange("p (h c) -> p h c", h=HEADS_PER_CORE),
                    )

            # ---- phase 3+4: attention per head pair, out-proj per group ----
            with (
                tc.tile_pool(name="onorm", bufs=1) as onorm_p,
                tc.tile_pool(name="ep", bufs=3) as e_p,
                tc.tile_pool(name="bc", bufs=2) as bc_p,
                tc.tile_pool(name="outsb", bufs=2) as out_p,
                tc.tile_pool(name="psS", bufs=2, space="PSUM") as psS,
                tc.tile_pool(name="psO", bufs=1, space="PSUM") as psO,
            ):
                onorm = onorm_p.tile([P, 2, T], f32r)

                for g in range(NG):
                    njt = 4 * g + 4  # causal k-tiles for this query group
                    for hp in range(2):  # head pairs (0,1) and (2,3)
                        o_ps = {}
                        for a in range(2):  # head within pair -> partitions 64a..
                            o_ps[a] = psO.tile([D + 1, 512], f32, tag=f"o{a}", name=f"o_ps{a}")
                        for jt in range(njt):
                            cs = P * (jt - 4 * g) if jt >= 4 * g else 0
                            e_t = {}
                            for a in range(2):
                                h = 2 * hp + a
                                s_ps = psS.tile([P, 512], f32, tag=f"s{a}")
                                nc.tensor.matmul(
                                    s_ps[:],
                                    qkT[64 * a : 64 * a + 64, 2 + hp, P * jt : P * (jt + 1)],
                                    qkT[64 * a : 64 * a + 64, hp, 512 * g : 512 * (g + 1)],
                                    start=True,
                                    stop=True,
                                )
                                e_t[a] = e_p.tile([P, 512], f32r, tag=f"e{a}", name=f"e_t{a}")
                                nc.scalar.activation(
                                    e_t[a][:, cs:512],
                                    s_ps[:, cs:512],
                                    AF.Exp,
                                    scale=float(D) ** -0.5,
                                )
                                if jt >= 4 * g:
                                    nc.vector.tensor_tensor(
                                        e_t[a][:, cs : cs + P],
                                        e_t[a][:, cs : cs + P],
                                        tri_sb[:],
                                        OP.mult,
                                    )
                            for a in range(2):
                                h = 2 * hp + a
                                nc.tensor.matmul(
                                    o_ps[a][:, cs:512],
                                    v_sb[:, jt, (D + 1) * h : (D + 1) * (h + 1)],
                                    e_t[a][:, cs:512],
                                    start=(jt == 0),
                                    stop=(jt == njt - 1),
                                )
                        # normalize: recip of sums row, broadcast via K=1 matmul
                        for a in range(2):
                            rec = bc_p.tile([1, 512], f32r, tag="rec")
                            nc.vector.reciprocal(rec[:], o_ps[a][D : D + 1, :])
                            bc_ps = psS.tile([D, 512], f32, tag=f"s{a}")
                            nc.tensor.matmul(
                                bc_ps[:], ones_sb[:], rec[:], start=True, stop=True
                            )
                            bc_sb = bc_p.tile([D, 512], f32, tag="bc")
                            nc.vector.tensor_copy(bc_sb[:], bc_ps[:])
                            nc.vector.tensor_tensor(
                                onorm[64 * a : 64 * a + 64, hp, 512 * g : 512 * (g + 1)],
                                o_ps[a][:D, :],
                                bc_sb[:],
                                OP.mult,
                            )

                    # ---- out-projection for this query group ----
                    out_sb = out_p.tile([P, 4, EMB], f32)
                    for s in range(4):
                        i = 4 * g + s
                        for n in range(2):
                            po = psP.tile([P, 512], f32, tag="pp")
                            for p2 in range(2):
                                nc.tensor.matmul(
                                    po[:],
                                    onorm[:, p2, P * i : P * (i + 1)],
                                    wout_sb[:, p2, 512 * n : 512 * (n + 1)],
                                    start=(p2 == 0),
                                    stop=(p2 == 1),
                                )
                            nc.vector.tensor_copy(
                                out_sb[:, s, 512 * n : 512 * (n + 1)], po[:]
                            )
                    nc.sync.dma_start(
                        out_d[512 * g : 512 * (g + 1), :].rearrange(
                            "(s p) n -> p s n", p=P
                        ),
                        out_sb[:],
                    )

    nc.compile()
    return nc


def kernel(x, Wqkv, bqkv, Wout, bout):
    from concourse.bass_utils import run_bass_kernel_spmd

    x = np.asarray(x, dtype=np.float32)
    Wqkv = np.asarray(Wqkv, dtype=np.float32)
    bqkv = np.asarray(bqkv, dtype=np.float32)
    Wout = np.asarray(Wout, dtype=np.float32)
    bout = np.asarray(bout, dtype=np.float32)

    if "nc" not in _prog_cache:
        _prog_cache["nc"] = _build_program()
    nc = _prog_cache["nc"]

    trimask = np.triu(np.ones((P, P), dtype=np.float32))
    xT = [np.ascontiguousarray(x[b].T) for b in range(B)]

    in_maps = []
    for c in range(N_CORES):
        b, m = divmod(c, TP)
        q0 = FSH * m
        wqk = np.concatenate(
            [Wqkv[:, q0 : q0 + FSH], Wqkv[:, H * D + q0 : H * D + q0 + FSH]], axis=1
        )
        wv = np.ascontiguousarray(Wqkv[:, 2 * H * D + q0 : 2 * H * D + q0 + FSH])
        wout = np.ascontiguousarray(Wout[q0 : q0 + FSH, :])
        bqk = np.concatenate(
            [bqkv[q0 : q0 + FSH], bqkv[H * D + q0 : H * D + q0 + FSH]]
        )
        in_maps.append(
            {
                "xT": xT[b],
                "wqk": np.ascontiguousarray(wqk),
                "wv": wv,
                "wout": wout,
                "bqk": np.ascontiguousarray(bqk.reshape(4, P).T),
                "trimask": trimask,
            }
        )

    res = run_bass_kernel_spmd(nc, in_maps, core_ids=list(range(N_CORES)))
    _prog_cache["last_result"] = res

    # v-bias contribution folded into the output bias (attn rows sum to 1)
    bout_eff = bout + bqkv[2 * H * D :] @ Wout

    out = np.empty((B, T, EMB), dtype=np.float32)
    for b in range(B):
        acc = res.results[TP * b]["out"].astype(np.float32).copy()
        for m in range(1, TP):
            acc += res.results[TP * b + m]["out"]
        out[b] = acc + bout_eff
    return out
